# revision 9
# baseline (speedup 1.0000x reference)
"""CrossViewTransformer kernel for 8 Trainium2 NeuronCores (one batch/core).

Math (per batch element, n = H*W = 4096):
    q = wq @ xq + bq            [8, n]
    k = wk @ xr + bk            [8, n]
    v = wv @ xr + bv            [64, n]
    energy[j, i] = sum_p k[p, j] q[p, i]
    att = softmax(energy, axis=-1)          (softmax over i)
    z[c, j] = sum_i v[c, i] att[j, i]
    out = xq + z

Design (v2): the softmax exp over 16.8M energies is the hard floor, so it
is split across TWO engines concurrently (GPSIMD cannot read PSUM):
  * ACT: real exp on full-t [128,1024] chunks (amortizes its fixed
    overhead), writing fp8e4m3 directly.
  * DVE: Schraudolph bit-trick exp on [128,512] chunks -- one
    tensor_scalar (i8 = round(e*8*log2e + C2)) whose int8 output IS the
    fp8e4m3 encoding of ~exp(e). Verified on HW: round-to-nearest, ~2.7%
    rel err; harmless here (z contributes only 0.7% of output norm; the
    residual path stays exact fp32).
  * Softmax denominator falls out of the z matmul via a ones-column in
    v^T (energy computed TRANSPOSED eT[i, j], softmax over partitions).
Energy flows through a manually-rotated 6x512 PSUM ring inside ONE tile
so an ACT chunk can span two adjacent slots; sub-tile range tracking
provides the RAW/WAR sync. All matmuls run fp8 DoubleRow (2 k-tiles per
instr, 0.5 cyc/row):
  * energy: contraction 8 padded to 2x8 with a zeros slot.
  * z: pairs of i-tiles, vt blocks stored at 16B-aligned stride 80.
PSUM: 6-bank energy ring + 2-bank z accumulation (JBW=1024, 4 j-blocks).
Finalize reads z straight from PSUM: 1/s on DVE (fp16), broadcast via a
tiny ones matmul into borrowed ring slots, mul (DVE) + residual add
(GPSIMD, SBUF-only) against fp32 query_x.
"""

import sys

if "/opt/trn_rl_repo" not in sys.path:
    sys.path.insert(0, "/opt/trn_rl_repo")

from contextlib import ExitStack

import numpy as np
import ml_dtypes

import concourse.tile as tile
from concourse import bacc, mybir
from concourse.bass_utils import run_bass_kernel_spmd

B = 8
C = 64
HW = 4096
PROJ = 8
NCORES = 8

F32 = mybir.dt.float32
BF16 = mybir.dt.bfloat16
F16 = mybir.dt.float16
FP8 = mybir.dt.float8e4
I8 = mybir.dt.int8
EXP = mybir.ActivationFunctionType.Exp
COPYF = mybir.ActivationFunctionType.Copy
MULT = mybir.AluOpType.mult
ADD = mybir.AluOpType.add
DR = mybir.MatmulPerfMode.DoubleRow

NT = HW // 128          # 32 i-tiles
JBW = 1024              # j-block width (z psum = 2 banks)
NJB = HW // JBW         # 4
CW = 512                # ring slot width (1 psum bank)
NSLOT = 6
NPAIR = NT // 2         # 16 z DoubleRow pairs
VTS = 80                # vt block stride (16B-aligned, 65 used)

LOG2E = float(np.log2(np.e))
EC1 = 8.0 * LOG2E       # exp trick: i8 = round(e*EC1 + EC2)
EC2 = 56.0 - 0.344

# measured per-512-chunk engine costs (ns): ACT 574 (paired), DVE 420
W_ACT, W_DVE = 1.74, 2.20


def _mk_schedule(n, ws):
    err = [0.0] * len(ws)
    tot = sum(ws)
    out = []
    for _ in range(n):
        for i in range(len(ws)):
            err[i] += ws[i] / tot
        p = max(range(len(ws)), key=lambda i: err[i])
        err[p] -= 1.0
        out.append(p)
    return out


def _build_nc():
    nc = bacc.Bacc("TRN2", target_bir_lowering=False, debug=False, num_devices=NCORES)

    xqb_d = nc.dram_tensor("xqb", [C + 1, HW], BF16, kind="ExternalInput").ap()
    xrb_d = nc.dram_tensor("xrb", [C + 1, HW], BF16, kind="ExternalInput").ap()
    xqf_d = nc.dram_tensor("xqf", [C, HW], F32, kind="ExternalInput").ap()
    wq_d = nc.dram_tensor("wq", [C + 1, PROJ], BF16, kind="ExternalInput").ap()
    wk_d = nc.dram_tensor("wk", [C + 1, PROJ], BF16, kind="ExternalInput").ap()
    wv_d = nc.dram_tensor("wv", [C + 1, C + 1], BF16, kind="ExternalInput").ap()
    out_d = nc.dram_tensor("out", [C, HW], F32, kind="ExternalOutput").ap()
    rs_d = nc.dram_tensor("rscratch", [NJB, JBW], F32).ap()

    with tile.TileContext(nc) as tc, ExitStack() as ctx:
        sgl = ctx.enter_context(tc.tile_pool(name="sgl", bufs=1))
        rpool = ctx.enter_context(tc.tile_pool(name="rng", bufs=1, space="PSUM"))
        zpool = ctx.enter_context(tc.tile_pool(name="zp", bufs=1, space="PSUM"))
        ntpool = ctx.enter_context(tc.tile_pool(name="nt", bufs=3))
        fpool = ctx.enter_context(tc.tile_pool(name="fin", bufs=2))

        xqb = sgl.tile([C + 1, HW], BF16)
        xrb = sgl.tile([C + 1, HW], BF16)
        xqf = sgl.tile([C, HW], F32)
        wq_sb = sgl.tile([C + 1, PROJ], BF16)
        wk_sb = sgl.tile([C + 1, PROJ], BF16)
        wv_sb = sgl.tile([C + 1, C + 1], BF16)
        q8 = sgl.tile([PROJ, 2 * HW], FP8)   # [:, :HW] data, [:, HW:] zeros
        k8 = sgl.tile([PROJ, 2 * HW], FP8)
        vt8 = sgl.tile([128, NT * VTS], FP8)  # 32 blocks of 65 @ stride 80

        ring = rpool.tile([128, NSLOT * CW], F32)  # 6-bank energy ring
        slot_i = [0]

        def slot(n=1):
            """Claim n adjacent ring slots (n<=2, no wrap); returns col base."""
            s = slot_i[0] % NSLOT
            if s + n > NSLOT:
                slot_i[0] += NSLOT - s  # skip wrap gap
                s = 0
            slot_i[0] += n
            return s * CW

        # zero pads + ones first (no deps; hidden under input DMA)
        nc.vector.memset(q8[:, HW : 2 * HW], 0.0)
        nc.gpsimd.memset(k8[:, HW : 2 * HW], 0.0)

        nc.sync.dma_start(out=wq_sb[:, :], in_=wq_d[:, :])
        nc.sync.dma_start(out=wk_sb[:, :], in_=wk_d[:, :])
        nc.sync.dma_start(out=wv_sb[:, :], in_=wv_d[:, :])
        NLC = 4
        LCW = HW // NLC
        for ci in range(NLC):
            nc.sync.dma_start(
                out=xrb[:, ci * LCW : (ci + 1) * LCW],
                in_=xrb_d[:, ci * LCW : (ci + 1) * LCW],
            )
        for ci in range(NLC):
            nc.sync.dma_start(
                out=xqb[:, ci * LCW : (ci + 1) * LCW],
                in_=xqb_d[:, ci * LCW : (ci + 1) * LCW],
            )
        nc.sync.dma_start(out=xqf[:, :], in_=xqf_d[:, :])

        q3 = q8[:, :].rearrange("p (i n) -> p i n", i=2)
        k3 = k8[:, :].rearrange("p (i n) -> p i n", i=2)

        # ---- helper jobs ------------------------------------------------
        ev_sched = _mk_schedule(21, [1.0, 1.2])  # proj/vt evac engines
        ev_i = [0]

        def evac(out_ap, in_ap):
            e = ev_sched[ev_i[0] % len(ev_sched)]
            ev_i[0] += 1
            if e == 0:
                nc.scalar.activation(out=out_ap, in_=in_ap, func=COPYF)
            else:
                nc.vector.tensor_copy(out=out_ap, in_=in_ap)

        def emit_proj(which, c):
            w_sb, x_sb, dst = {
                "q": (wq_sb, xqb, q8),
                "k": (wk_sb, xrb, k8),
            }[which]
            s0 = slot()
            nc.tensor.matmul(
                ring[0:PROJ, s0 : s0 + CW],
                lhsT=w_sb[:, :],
                rhs=x_sb[:, c * CW : (c + 1) * CW],
                start=True,
                stop=True,
            )
            evac(dst[0:PROJ, c * CW : (c + 1) * CW], ring[0:PROJ, s0 : s0 + CW])

        def emit_vt(g):
            nb = 7 if g < 4 else 4  # 4*7 + 4 = 32 blocks
            s0 = slot()
            for j in range(nb):
                t = g * 7 + j
                nc.tensor.matmul(
                    ring[:, s0 + j * 65 : s0 + (j + 1) * 65],
                    lhsT=xrb[:, t * 128 : (t + 1) * 128],
                    rhs=wv_sb[:, :],
                    start=True,
                    stop=True,
                )
            evac(
                vt8[:, g * 7 * VTS : (g * 7 + nb) * VTS].rearrange(
                    "p (t m) -> p t m", t=nb
                )[:, :, 0:65],
                ring[:, s0 : s0 + nb * 65].rearrange("p (t m) -> p t m", t=nb),
            )

        # exp-engine schedule: 0=ACT (full-t pair), 1=DVE (two singles)
        xsched = _mk_schedule(NJB * NT * 2, [W_ACT, W_DVE])

        def emit_energy_exp(jb, t, c0, nt_t):
            """Emit energy+exp for chunks (t,c0[,c0+1]) -- c0=0 may pair."""
            ci = (jb * NT + t) * 2
            pair = c0 == 0 and xsched[ci] == 0 and xsched[ci + 1] == 0
            n = 2 if pair else 1
            s0 = slot(n)
            j0 = jb * JBW + c0 * CW
            for i in range(n):
                nc.tensor.matmul(
                    ring[:, s0 + i * CW : s0 + (i + 1) * CW],
                    lhsT=q3[:, :, t * 128 : (t + 1) * 128],
                    rhs=k3[:, :, j0 + i * CW : j0 + (i + 1) * CW],
                    start=True,
                    stop=True,
                    perf_mode=DR,
                )
            dst = nt_t[:, (t % 2) * JBW + c0 * CW : (t % 2) * JBW + (c0 + n) * CW]
            if xsched[ci + c0] == 0:
                nc.scalar.activation(
                    out=dst, in_=ring[:, s0 : s0 + n * CW], func=EXP
                )
            else:
                nc.vector.tensor_scalar(
                    out=dst.bitcast(I8), in0=ring[:, s0 : s0 + n * CW],
                    scalar1=EC1, scalar2=EC2, op0=MULT, op1=ADD,
                )
            return n

        def emit_z(p, nt_t, zps):
            vpair = vt8[:, p * 2 * VTS : (p * 2 + 2) * VTS].rearrange(
                "p (i m) -> p i m", i=2
            )[:, :, 0:65]
            n3 = nt_t[:, :].rearrange("p (i n) -> p i n", i=2)
            for c in range(2):
                nc.tensor.matmul(
                    zps[:, c * CW : (c + 1) * CW],
                    lhsT=vpair,
                    rhs=n3[:, :, c * CW : (c + 1) * CW],
                    start=(p == 0),
                    stop=(p == NPAIR - 1),
                    perf_mode=DR,
                )

        # ---- prologue projections (enough for jb0 start) ----------------
        emit_proj("k", 0)
        emit_proj("q", 0)
        emit_proj("k", 1)
        for c in range(1, 4):
            emit_proj("q", c)

        # ---- main loop --------------------------------------------------
        for jb in range(NJB):
            zps = zpool.tile([65, JBW], F32, tag="z", name=f"z{jb}")
            nts = [None] * NPAIR

            for t in range(NT):
                if t % 2 == 0:
                    nts[t // 2] = ntpool.tile(
                        [128, 2 * JBW], FP8, tag="n", name=f"nt{jb}_{t}"
                    )
                nt_t = nts[t // 2]
                done = emit_energy_exp(jb, t, 0, nt_t)
                if done == 1:
                    emit_energy_exp(jb, t, 1, nt_t)
                # trailing setup work woven into jb0
                if jb == 0:
                    if t in (0, 2, 4, 6, 8):
                        emit_vt(t // 2)
                    if t in (10, 12, 14, 16):
                        emit_proj("q", 4 + (t - 10) // 2)
                    if t in (18, 20):
                        emit_proj("k", 2 + (t - 18) // 2)
                elif jb < NJB - 1 and t in (26, 28):
                    emit_proj("k", 2 * (jb + 1) + (t - 26) // 2)
                if t % 2 == 1 and t >= 3:
                    p = (t - 1) // 2 - 1
                    emit_z(p, nts[p], zps)
            emit_z(NPAIR - 1, nts[NPAIR - 1], zps)

            # ---- finalize: out = xq + z/s, z read straight from PSUM ----
            # 1/s broadcast over partitions via a DRAM bounce (DMA
            # partition-step-0 source is DRAM-only).
            j0 = jb * JBW
            rr = fpool.tile([1, JBW], F32, tag="r")
            nc.vector.reciprocal(out=rr[:, :], in_=zps[C : C + 1, :])
            nc.sync.dma_start(out=rs_d[jb, :], in_=rr[:, :])
            rb_sb = fpool.tile([C, JBW], F32, tag="rb")
            nc.sync.dma_start(
                out=rb_sb[:, :], in_=rs_d[jb : jb + 1, :].partition_broadcast(C)
            )
            o_sb = fpool.tile([C, JBW], F32, tag="o")
            nc.vector.tensor_mul(o_sb[:, 0:CW], zps[0:C, 0:CW], rb_sb[:, 0:CW])
            nc.vector.tensor_mul(
                o_sb[:, CW:JBW], zps[0:C, CW:JBW], rb_sb[:, CW:JBW]
            )
            nc.gpsimd.tensor_add(
                o_sb[:, 0:CW], o_sb[:, 0:CW], xqf[:, j0 : j0 + CW]
            )
            nc.gpsimd.tensor_add(
                o_sb[:, CW:JBW], o_sb[:, CW:JBW], xqf[:, j0 + CW : j0 + JBW]
            )
            nc.sync.dma_start(out=out_d[:, j0 : j0 + JBW], in_=o_sb[:, :])

    nc.compile()
    return nc


_NC = None


def _get_nc():
    global _NC
    if _NC is None:
        _NC = _build_nc()
    return _NC


def _make_in_maps(query_x, ref_x, wq, bq, wk, bk, wv, bv):
    query_x = np.ascontiguousarray(np.asarray(query_x, dtype=np.float32))
    ref_x = np.ascontiguousarray(np.asarray(ref_x, dtype=np.float32))
    wq = np.asarray(wq, dtype=np.float32)
    bq = np.asarray(bq, dtype=np.float32)
    wk = np.asarray(wk, dtype=np.float32)
    bk = np.asarray(bk, dtype=np.float32)
    wv = np.asarray(wv, dtype=np.float32)
    bv = np.asarray(bv, dtype=np.float32)

    # lhsT layouts: [in_ch(+bias row), out_ch]
    wq_a = np.concatenate([wq.T, bq[None, :]], axis=0).astype(ml_dtypes.bfloat16)
    wk_a = np.concatenate([wk.T, bk[None, :]], axis=0).astype(ml_dtypes.bfloat16)
    wv_a = np.zeros((C + 1, C + 1), dtype=np.float32)
    wv_a[:C, :C] = wv.T
    wv_a[C, :C] = bv
    wv_a[C, C] = 1.0  # ones column -> softmax denominator row of z psum
    wv_a = wv_a.astype(ml_dtypes.bfloat16)

    ones = np.ones((1, HW), dtype=np.float32)
    in_maps = []
    for b in range(B):
        xq = query_x[b].reshape(C, HW)
        xr = ref_x[b].reshape(C, HW)
        xq_a = np.concatenate([xq, ones], axis=0).astype(ml_dtypes.bfloat16)
        xr_a = np.concatenate([xr, ones], axis=0).astype(ml_dtypes.bfloat16)
        in_maps.append(
            {
                "xqb": np.ascontiguousarray(xq_a),
                "xrb": np.ascontiguousarray(xr_a),
                "xqf": np.ascontiguousarray(xq),
                "wq": wq_a,
                "wk": wk_a,
                "wv": wv_a,
            }
        )
    return in_maps


def kernel(query_x, ref_x, wq, bq, wk, bk, wv, bv):
    nc = _get_nc()
    in_maps = _make_in_maps(query_x, ref_x, wq, bq, wk, bk, wv, bv)
    res = run_bass_kernel_spmd(nc, in_maps, core_ids=list(range(NCORES)))
    out = np.stack([r["out"].reshape(C, 64, 64) for r in res.results], axis=0)
    return np.ascontiguousarray(out.astype(np.float32))
